# revision 41
# baseline (speedup 1.0000x reference)
"""Trainium2 Bass kernel for nn_DepthMemoryCache.

Reference computation (D=8, B=4, S=4096, C=1024, G=64):
    u     = einsum('bsc,gc->bsg', x[-1], W_u)
    keys  = einsum('dbc,gc->dbg', x.mean(2), W_u)
    gates = softmax(einsum('bsg,dbg->bsd', u, keys), axis=-1)
    out   = einsum('dbsc,bsd->bsc', x, gates)

Strategy: shard the sequence axis over 8 cores (core i gets
x[:, :, i*512:(i+1)*512, :]). Per core, two streaming passes over the 64MB
shard:
  A) depth/batch sums over s on PE: slabs are cast to bf16 (on the otherwise
     idle DVE/ACT engines) and column-summed with indicator stationaries in a
     single PSUM accumulation region. For the d=D-1 slabs, uT = W_u @ x7.T is
     also computed on PE (bf16 transposes + matmuls) so phase B needs no
     per-block transposes. A 128KB all-core AllReduce completes the
     full-sequence means (a tiny warm-up AllReduce at kernel start absorbs
     comm setup under phase A; collective bounce DMAs ride GpSimd's queue so
     the Sync engine keeps issuing prefetch reads).
  B) after a short fixup (meanT transposes + keysT matmuls), each 128-row
     block needs ONE small matmul for logits, softmax via ACT exp with
     accum_out, then 8 streamed depth tiles combined by fused
     scalar_tensor_tensor FMAs (fp32, exact) with per-partition gate scalars
     on DVE; gates are interleaved with streaming so the first FMA fires
     right after the collective.
HBM traffic per core: 64 (A) + 64 (B) + 8 (write) = 136MB.
The bf16 mean/logit paths cost ~1e-3/2e-4 relative on gates only; the output
weighted sum stays fp32.
"""
import sys

sys.path.insert(0, "/opt/trn_rl_repo")

from contextlib import ExitStack

import numpy as np
from concourse import bacc, bass, mybir, tile, masks
from concourse import bass_utils

F32 = mybir.dt.float32
BF16 = mybir.dt.bfloat16

D, B, S, C, G = 8, 4, 4096, 1024, 64
N_CORES = 8
P = 128                 # partition count / block rows
NKC = C // P            # 8 column chunks of 128


def build_body(tc, x, w, y, s_sh):
    """Emit the kernel IR. x:[D,B,s_sh,C], w:[G,C], y:[B,s_sh,C] dram APs."""
    nc = tc.nc
    nj = s_sh // P      # 128-row blocks per (d, b)
    mul, add = mybir.AluOpType.mult, mybir.AluOpType.add
    DB = D * B
    es = ExitStack()

    singles = es.enter_context(tc.tile_pool(name="singles", bufs=1))
    ident = singles.tile([P, P], F32)
    masks.make_identity(nc, ident[:])
    ident_bf = singles.tile([P, P], BF16)
    masks.make_identity(nc, ident_bf[:])
    # indicator stationaries: ind[:, r, m] = (m == r) / S  — column-sums a
    # bf16 slab into psum row r with one N=512 matmul per c-half.
    ind_bf = singles.tile([P, DB, DB], BF16)
    nc.vector.memset(ind_bf[:], 0.0)
    for r in range(DB):
        nc.vector.memset(ind_bf[:, r, r:r + 1], 1.0 / (N_CORES * s_sh))
    w_sb = singles.tile([G, C], F32)
    nc.sync.dma_start(w_sb[:], w[:])
    gates_sb = singles.tile([P, B, nj, D], F32)
    sums_sb = singles.tile([DB, C], F32)
    sumk_sb = singles.tile([G, B * D], F32)
    meanT_sb = singles.tile([P, NKC * DB], F32)
    wT_sb = singles.tile([P, NKC, G], F32)
    wT_bf = singles.tile([P, NKC, G], BF16)
    keysT_sb = singles.tile([G, B, D], F32)
    uT_sb = singles.tile([G, B, nj, P], F32)

    stream = es.enter_context(tc.tile_pool(name="stream", bufs=3))
    bfp = es.enter_context(tc.tile_pool(name="bfp", bufs=3))

    dram = es.enter_context(tc.tile_pool(name="dram", bufs=1, space="DRAM"))
    # tiny warm-up AllReduce: absorbs collective-comm setup under phase A
    ccw_in = dram.tile([1, 16], F32)
    ccw_out = dram.tile([1, 16], F32)
    cc_in = dram.tile([G, B * D], F32)
    cc_out = dram.tile([G, B * D], F32)
    warm_sb = singles.tile([1, 16], F32)
    nc.vector.memset(warm_sb[:], 0.0)
    nc.gpsimd.dma_start(ccw_in[:], warm_sb[:])
    nc.gpsimd.collective_compute(
        "AllReduce", add, replica_groups=[list(range(N_CORES))],
        ins=[ccw_in.opt()], outs=[ccw_out.opt()],
    )

    # ---------------- Phase A: partial sums over s (scaled by 1/S) ----------
    with tc.tile_pool(name="psumA", bufs=1, space="PSUM") as psA, \
         tc.tile_pool(name="psumT", bufs=1, space="PSUM") as psT, \
         tc.tile_pool(name="psumXA", bufs=3, space="PSUM") as psXA, \
         tc.tile_pool(name="psumU", bufs=2, space="PSUM") as psU, \
         tc.tile_pool(name="xtA", bufs=3) as xtA:
        sums_ps = psA.tile([DB, C], F32)

        # Each 512-col half of sums_ps is one 2KB PSUM zero region: start=True
        # zeroes the WHOLE region, so exactly one start (global first MM into
        # that region) / one stop (global last); every other matmul
        # accumulates onto pending-zero bytes. Rows m != r get +0.
        def sum_slab(slab_bf, d, b, first, last):
            r = d * B + b
            for h in range(2):
                for j in range(nj):
                    nc.tensor.matmul(
                        sums_ps[:, h * 512:(h + 1) * 512],
                        ind_bf[:, r, :],
                        slab_bf[:, j, h * 512:(h + 1) * 512],
                        start=(first and j == 0),
                        stop=(last and j == nj - 1),
                    )

        def cast_slab(dst_bf, src_f32, i):
            # split the fp32->bf16 casts between DVE and ACT (both idle here)
            for j in range(nj):
                if (i * nj + j) % 2 == 0:
                    nc.vector.tensor_copy(dst_bf[:, j, :], src_f32[:, j, :])
                else:
                    nc.scalar.copy(dst_bf[:, j, :], src_f32[:, j, :])

        # one-time W_u transpose: wT[c, g] chunks (fp32 + bf16 copies)
        for k in range(NKC):
            tr = psT.tile([P, NKC * DB], F32, tag="fix")
            nc.tensor.transpose(tr[:, :G], w_sb[:, k * P:(k + 1) * P], ident[:G, :G])
            nc.vector.tensor_copy(wT_sb[:, k, :], tr[:, :G])
            nc.scalar.copy(wT_bf[:, k, :], tr[:, :G])

        for d in range(D):
            for b in range(B):
                slab = stream.tile([P, nj, C], F32, tag="slab")
                nc.sync.dma_start(
                    slab[:], x[d, b].rearrange("(j p) c -> p j c", p=P))
                xbf_t = bfp.tile([P, nj, C], BF16, tag="xbf")
                xbf = xbf_t[:]
                cast_slab(xbf, slab[:], d * B + b)
                sum_slab(xbf, d, b, first=(d == 0 and b == 0),
                         last=(d == D - 1 and b == B - 1))
                if d == D - 1:
                    # uT[g, s-block] = sum_k (wT_k).T @ x7T_k on PE
                    for j in range(nj):
                        u_ps = psU.tile([G, P], F32, tag="u")
                        for k in range(NKC):
                            xt_ps = psXA.tile([P, P], BF16, tag="xt_ps")
                            nc.tensor.transpose(
                                xt_ps[:], xbf[:, j, k * P:(k + 1) * P],
                                ident_bf[:])
                            xt_sb = xtA.tile([P, P], BF16, tag="xt_sb")
                            if k % 2 == 0:
                                nc.scalar.copy(xt_sb[:], xt_ps[:])
                            else:
                                nc.vector.tensor_copy(xt_sb[:], xt_ps[:])
                            nc.tensor.matmul(
                                u_ps[:], wT_bf[:, k, :], xt_sb[:],
                                start=(k == 0), stop=(k == NKC - 1))
                        nc.vector.tensor_copy(uT_sb[:, b, j, :], u_ps[:])

        nc.vector.tensor_copy(sums_sb[:], sums_ps[:])

        # ---- local partial keysT (keys are linear in the means, so the ----
        # ---- AllReduce can run in the tiny keys space: 8KB not 128KB)  ----
        # meanT[c, (d,b)] chunks via PE transpose — all 8 into one psum tile
        # (one zero region => single start/stop accumulation group)
        mt_ps = psT.tile([P, NKC * DB], F32, tag="fix")
        for k in range(NKC):
            nc.tensor.matmul(
                mt_ps[:, k * DB:(k + 1) * DB],
                sums_sb[:, k * P:(k + 1) * P], ident[:DB, :DB],
                is_transpose=True, start=(k == 0), stop=(k == NKC - 1))
        nc.vector.tensor_copy(meanT_sb[:], mt_ps[:])
        # partial keysT[g, d] per b = sum_k wT_k.T @ meanT_k
        keys_ps = psT.tile([P, NKC * DB], F32, tag="fix")
        for b in range(B):
            for k in range(NKC):
                nc.tensor.matmul(
                    keys_ps[:G, b * D:(b + 1) * D],
                    wT_sb[:, k, :],
                    meanT_sb[:, k * DB:(k + 1) * DB].rearrange(
                        "p (d b) -> p d b", b=B)[:, :, b],
                    start=(k == 0), stop=(k == NKC - 1),
                )
        nc.vector.tensor_copy(sumk_sb[:], keys_ps[:G, :B * D])

    # ---------------- AllReduce the [G, B*D] partial keys -------------------
    # bounce DMAs go through GpSimd's queue so the Sync engine never blocks
    # on the collective and keeps issuing phase-B prefetch reads.
    nc.gpsimd.dma_start(cc_in[:], sumk_sb[:])
    nc.gpsimd.collective_compute(
        "AllReduce", add,
        replica_groups=[list(range(N_CORES))],
        ins=[cc_in.opt()], outs=[cc_out.opt()],
    )
    nc.gpsimd.dma_start(
        keysT_sb[:].rearrange("g b d -> g (b d)"), cc_out[:])

    # ---------------- Phase B: gates + depth-weighted sum -------------------
    with tc.tile_pool(name="psumL", bufs=2, space="PSUM") as psL, \
         tc.tile_pool(name="bstream", bufs=16) as bstream, \
         tc.tile_pool(name="accp", bufs=4) as accp, \
         tc.tile_pool(name="small", bufs=4) as small:
        for b in range(B):
            for j in range(nj):
                # logits for this block: one small matmul off resident uT
                lg_ps = psL.tile([P, D], F32, tag="lg")
                nc.tensor.matmul(lg_ps[:], uT_sb[:, b, j, :], keysT_sb[:, b, :])
                e_sb = small.tile([P, D], F32, tag="e")
                z_sb = small.tile([P, 1], F32, tag="z")
                rz_sb = small.tile([P, 1], F32, tag="rz")
                nc.scalar.activation(
                    e_sb[:], lg_ps[:], mybir.ActivationFunctionType.Exp,
                    accum_out=z_sb[:])
                nc.vector.reciprocal(rz_sb[:], z_sb[:])
                nc.scalar.mul(gates_sb[:, b, j, :], e_sb[:], rz_sb[:])

                acc = accp.tile([P, C], F32, tag="acc")
                for dd in range(D):
                    d = (dd + D - 1) % D        # d = 7 first, then 0..6
                    t = bstream.tile([P, C], F32, tag="bslab")
                    nc.sync.dma_start(
                        t[:], x[d, b, j * P:(j + 1) * P, :])
                    if dd == 0:
                        nc.scalar.mul(
                            acc[:], t[:], gates_sb[:, b, j, d:d + 1])
                    else:
                        nc.vector.scalar_tensor_tensor(
                            out=acc[:], in0=t[:],
                            scalar=gates_sb[:, b, j, d:d + 1],
                            in1=acc[:], op0=mul, op1=add)
                # y writes on ScalarE's HWDGE ring: keeps Sync's in-order
                # queue free for prefetch reads
                nc.scalar.dma_start(y[b, j * P:(j + 1) * P, :], acc[:])

    es.close()


def build_nc(s_sh):
    nc = bacc.Bacc("TRN2", target_bir_lowering=False, debug=False,
                   num_devices=N_CORES)
    x_ap = nc.dram_tensor("x", [D, B, s_sh, C], F32, kind="ExternalInput").ap()
    w_ap = nc.dram_tensor("w", [G, C], F32, kind="ExternalInput").ap()
    y_ap = nc.dram_tensor("y", [B, s_sh, C], F32, kind="ExternalOutput").ap()
    with tile.TileContext(nc) as tc:
        build_body(tc, x_ap, w_ap, y_ap, s_sh)
    nc.compile()
    return nc


_NC_CACHE = {}


def _get_nc(s_sh):
    if s_sh not in _NC_CACHE:
        _NC_CACHE[s_sh] = build_nc(s_sh)
    return _NC_CACHE[s_sh]


def run(cached_states, W_u, trace=False, trace_cores=None):
    s_sh = S // N_CORES
    nc = _get_nc(s_sh)
    xs = np.asarray(cached_states, dtype=np.float32)
    ws = np.ascontiguousarray(np.asarray(W_u, dtype=np.float32))
    in_maps = []
    for i in range(N_CORES):
        sh = np.ascontiguousarray(xs[:, :, i * s_sh:(i + 1) * s_sh, :])
        in_maps.append({"x": sh, "w": ws})
    res = bass_utils.run_bass_kernel_spmd(
        nc, in_maps, core_ids=list(range(N_CORES)), trace=trace,
        trace_cores=trace_cores)
    out = np.empty((B, S, C), np.float32)
    for i in range(N_CORES):
        out[:, i * s_sh:(i + 1) * s_sh, :] = res.results[i]["y"]
    return out, res


def kernel(cached_states, W_u):
    out, _ = run(cached_states, W_u)
    return out


# revision 43
# speedup vs baseline: 1.0122x; 1.0122x over previous
"""Trainium2 Bass kernel for nn_DepthMemoryCache.

Reference computation (D=8, B=4, S=4096, C=1024, G=64):
    u     = einsum('bsc,gc->bsg', x[-1], W_u)
    keys  = einsum('dbc,gc->dbg', x.mean(2), W_u)
    gates = softmax(einsum('bsg,dbg->bsd', u, keys), axis=-1)
    out   = einsum('dbsc,bsd->bsc', x, gates)

Strategy: shard the sequence axis over 8 cores (core i gets
x[:, :, i*512:(i+1)*512, :]). Per core, two streaming passes over the 64MB
shard:
  A) depth/batch sums over s on PE: slabs are cast to bf16 (on the otherwise
     idle DVE/ACT engines) and column-summed with indicator stationaries in a
     single PSUM accumulation region. For the d=D-1 slabs, uT = W_u @ x7.T is
     also computed on PE (bf16 transposes + matmuls) so phase B needs no
     per-block transposes. A 128KB all-core AllReduce completes the
     full-sequence means (a tiny warm-up AllReduce at kernel start absorbs
     comm setup under phase A; collective bounce DMAs ride GpSimd's queue so
     the Sync engine keeps issuing prefetch reads).
  B) after a short fixup (meanT transposes + keysT matmuls), each 128-row
     block needs ONE small matmul for logits, softmax via ACT exp with
     accum_out, then 8 streamed depth tiles combined by fused
     scalar_tensor_tensor FMAs (fp32, exact) with per-partition gate scalars
     on DVE; gates are interleaved with streaming so the first FMA fires
     right after the collective.
HBM traffic per core: 64 (A) + 64 (B) + 8 (write) = 136MB.
The bf16 mean/logit paths cost ~1e-3/2e-4 relative on gates only; the output
weighted sum stays fp32.
"""
import sys

sys.path.insert(0, "/opt/trn_rl_repo")

from contextlib import ExitStack

import numpy as np
from concourse import bacc, bass, mybir, tile, masks
from concourse import bass_utils

F32 = mybir.dt.float32
BF16 = mybir.dt.bfloat16

D, B, S, C, G = 8, 4, 4096, 1024, 64
N_CORES = 8
P = 128                 # partition count / block rows
NKC = C // P            # 8 column chunks of 128


def build_body(tc, x, w, y, s_sh):
    """Emit the kernel IR. x:[D,B,s_sh,C], w:[G,C], y:[B,s_sh,C] dram APs."""
    nc = tc.nc
    nj = s_sh // P      # 128-row blocks per (d, b)
    mul, add = mybir.AluOpType.mult, mybir.AluOpType.add
    DB = D * B
    es = ExitStack()

    singles = es.enter_context(tc.tile_pool(name="singles", bufs=1))
    ident = singles.tile([P, P], F32)
    masks.make_identity(nc, ident[:])
    ident_bf = singles.tile([P, P], BF16)
    masks.make_identity(nc, ident_bf[:])
    # indicator stationaries: ind[:, r, m] = (m == r) / S  — column-sums a
    # bf16 slab into psum row r with one N=512 matmul per c-half.
    ind_bf = singles.tile([P, DB, DB], BF16)
    nc.vector.memset(ind_bf[:], 0.0)
    for r in range(DB):
        nc.vector.memset(ind_bf[:, r, r:r + 1], 1.0 / (N_CORES * s_sh))
    w_sb = singles.tile([G, C], F32)
    nc.sync.dma_start(w_sb[:], w[:])
    gates_sb = singles.tile([P, B, nj, D], F32)
    sums_sb = singles.tile([DB, C], F32)
    sumk_sb = singles.tile([G, B * D], F32)
    meanT_sb = singles.tile([P, NKC * DB], F32)
    wT_sb = singles.tile([P, NKC, G], F32)
    wT_bf = singles.tile([P, NKC, G], BF16)
    keysT_sb = singles.tile([G, B, D], F32)
    uT_sb = singles.tile([G, B, nj, P], F32)

    stream = es.enter_context(tc.tile_pool(name="stream", bufs=3))
    bfp = es.enter_context(tc.tile_pool(name="bfp", bufs=3))

    dram = es.enter_context(tc.tile_pool(name="dram", bufs=1, space="DRAM"))
    # tiny warm-up AllReduce: absorbs collective-comm setup under phase A
    ccw_in = dram.tile([1, 16], F32)
    ccw_out = dram.tile([1, 16], F32)
    cc_in = dram.tile([G, B * D], F32)
    cc_out = dram.tile([G, B * D], F32)
    warm_sb = singles.tile([1, 16], F32)
    nc.vector.memset(warm_sb[:], 0.0)
    nc.gpsimd.dma_start(ccw_in[:], warm_sb[:])
    nc.gpsimd.collective_compute(
        "AllReduce", add, replica_groups=[list(range(N_CORES))],
        ins=[ccw_in.opt()], outs=[ccw_out.opt()],
    )

    # ---------------- Phase A: partial sums over s (scaled by 1/S) ----------
    with tc.tile_pool(name="psumA", bufs=1, space="PSUM") as psA, \
         tc.tile_pool(name="psumT", bufs=1, space="PSUM") as psT, \
         tc.tile_pool(name="psumXA", bufs=3, space="PSUM") as psXA, \
         tc.tile_pool(name="psumU", bufs=2, space="PSUM") as psU, \
         tc.tile_pool(name="xtA", bufs=3) as xtA:
        sums_ps = psA.tile([DB, C], F32)

        # Each 512-col half of sums_ps is one 2KB PSUM zero region: start=True
        # zeroes the WHOLE region, so exactly one start (global first MM into
        # that region) / one stop (global last); every other matmul
        # accumulates onto pending-zero bytes. Rows m != r get +0.
        def sum_slab(slab_bf, d, b, first, last):
            r = d * B + b
            for h in range(2):
                for j in range(nj):
                    nc.tensor.matmul(
                        sums_ps[:, h * 512:(h + 1) * 512],
                        ind_bf[:, r, :],
                        slab_bf[:, j, h * 512:(h + 1) * 512],
                        start=(first and j == 0),
                        stop=(last and j == nj - 1),
                    )

        def cast_slab(dst_bf, src_f32, i):
            # split the fp32->bf16 casts between DVE and ACT (both idle here)
            for j in range(nj):
                if (i * nj + j) % 2 == 0:
                    nc.vector.tensor_copy(dst_bf[:, j, :], src_f32[:, j, :])
                else:
                    nc.scalar.copy(dst_bf[:, j, :], src_f32[:, j, :])

        # one-time W_u transpose: wT[c, g] chunks (fp32 + bf16 copies)
        for k in range(NKC):
            tr = psT.tile([P, NKC * DB], F32, tag="fix")
            nc.tensor.transpose(tr[:, :G], w_sb[:, k * P:(k + 1) * P], ident[:G, :G])
            nc.vector.tensor_copy(wT_sb[:, k, :], tr[:, :G])
            nc.scalar.copy(wT_bf[:, k, :], tr[:, :G])

        for dd in range(D):
            d = (dd + D - 1) % D    # d = 7 first: its uT work overlaps the
            for b in range(B):      # remaining 7 depths' streaming on PE
                slab = stream.tile([P, nj, C], F32, tag="slab")
                nc.sync.dma_start(
                    slab[:], x[d, b].rearrange("(j p) c -> p j c", p=P))
                xbf_t = bfp.tile([P, nj, C], BF16, tag="xbf")
                xbf = xbf_t[:]
                cast_slab(xbf, slab[:], d * B + b)
                sum_slab(xbf, d, b, first=(dd == 0 and b == 0),
                         last=(dd == D - 1 and b == B - 1))
                if d == D - 1:
                    # uT[g, s-block] = sum_k (wT_k).T @ x7T_k on PE
                    for j in range(nj):
                        u_ps = psU.tile([G, P], F32, tag="u")
                        for k in range(NKC):
                            xt_ps = psXA.tile([P, P], BF16, tag="xt_ps")
                            nc.tensor.transpose(
                                xt_ps[:], xbf[:, j, k * P:(k + 1) * P],
                                ident_bf[:])
                            xt_sb = xtA.tile([P, P], BF16, tag="xt_sb")
                            if k % 2 == 0:
                                nc.scalar.copy(xt_sb[:], xt_ps[:])
                            else:
                                nc.vector.tensor_copy(xt_sb[:], xt_ps[:])
                            nc.tensor.matmul(
                                u_ps[:], wT_bf[:, k, :], xt_sb[:],
                                start=(k == 0), stop=(k == NKC - 1))
                        nc.vector.tensor_copy(uT_sb[:, b, j, :], u_ps[:])

        nc.vector.tensor_copy(sums_sb[:], sums_ps[:])

        # ---- local partial keysT (keys are linear in the means, so the ----
        # ---- AllReduce can run in the tiny keys space: 8KB not 128KB)  ----
        # meanT[c, (d,b)] chunks via PE transpose — all 8 into one psum tile
        # (one zero region => single start/stop accumulation group)
        mt_ps = psT.tile([P, NKC * DB], F32, tag="fix")
        for k in range(NKC):
            nc.tensor.matmul(
                mt_ps[:, k * DB:(k + 1) * DB],
                sums_sb[:, k * P:(k + 1) * P], ident[:DB, :DB],
                is_transpose=True, start=(k == 0), stop=(k == NKC - 1))
        nc.vector.tensor_copy(meanT_sb[:], mt_ps[:])
        # partial keysT[g, d] per b = sum_k wT_k.T @ meanT_k
        keys_ps = psT.tile([P, NKC * DB], F32, tag="fix")
        for b in range(B):
            for k in range(NKC):
                nc.tensor.matmul(
                    keys_ps[:G, b * D:(b + 1) * D],
                    wT_sb[:, k, :],
                    meanT_sb[:, k * DB:(k + 1) * DB].rearrange(
                        "p (d b) -> p d b", b=B)[:, :, b],
                    start=(k == 0), stop=(k == NKC - 1),
                )
        nc.vector.tensor_copy(sumk_sb[:], keys_ps[:G, :B * D])

    # ---------------- AllReduce the [G, B*D] partial keys -------------------
    # bounce DMAs go through GpSimd's queue so the Sync engine never blocks
    # on the collective and keeps issuing phase-B prefetch reads.
    nc.gpsimd.dma_start(cc_in[:], sumk_sb[:])
    nc.gpsimd.collective_compute(
        "AllReduce", add,
        replica_groups=[list(range(N_CORES))],
        ins=[cc_in.opt()], outs=[cc_out.opt()],
    )
    nc.gpsimd.dma_start(
        keysT_sb[:].rearrange("g b d -> g (b d)"), cc_out[:])

    # ---------------- Phase B: gates + depth-weighted sum -------------------
    with tc.tile_pool(name="psumL", bufs=2, space="PSUM") as psL, \
         tc.tile_pool(name="bstream", bufs=16) as bstream, \
         tc.tile_pool(name="accp", bufs=4) as accp, \
         tc.tile_pool(name="small", bufs=4) as small:
        for b in range(B):
            for j in range(nj):
                # logits for this block: one small matmul off resident uT
                lg_ps = psL.tile([P, D], F32, tag="lg")
                nc.tensor.matmul(lg_ps[:], uT_sb[:, b, j, :], keysT_sb[:, b, :])
                e_sb = small.tile([P, D], F32, tag="e")
                z_sb = small.tile([P, 1], F32, tag="z")
                rz_sb = small.tile([P, 1], F32, tag="rz")
                nc.scalar.activation(
                    e_sb[:], lg_ps[:], mybir.ActivationFunctionType.Exp,
                    accum_out=z_sb[:])
                nc.vector.reciprocal(rz_sb[:], z_sb[:])
                nc.scalar.mul(gates_sb[:, b, j, :], e_sb[:], rz_sb[:])

                acc = accp.tile([P, C], F32, tag="acc")
                for dd in range(D):
                    d = (dd + D - 1) % D        # d = 7 first, then 0..6
                    t = bstream.tile([P, C], F32, tag="bslab")
                    nc.sync.dma_start(
                        t[:], x[d, b, j * P:(j + 1) * P, :])
                    if dd == 0:
                        nc.vector.tensor_scalar_mul(
                            acc[:], t[:], gates_sb[:, b, j, d:d + 1])
                    else:
                        nc.vector.scalar_tensor_tensor(
                            out=acc[:], in0=t[:],
                            scalar=gates_sb[:, b, j, d:d + 1],
                            in1=acc[:], op0=mul, op1=add)
                # y writes via GpSimd (SWDGE): keeps both Sync's and ACT's
                # in-order queues free for prefetch reads / gate math
                nc.gpsimd.dma_start(y[b, j * P:(j + 1) * P, :], acc[:])

    es.close()


def build_nc(s_sh):
    nc = bacc.Bacc("TRN2", target_bir_lowering=False, debug=False,
                   num_devices=N_CORES)
    x_ap = nc.dram_tensor("x", [D, B, s_sh, C], F32, kind="ExternalInput").ap()
    w_ap = nc.dram_tensor("w", [G, C], F32, kind="ExternalInput").ap()
    y_ap = nc.dram_tensor("y", [B, s_sh, C], F32, kind="ExternalOutput").ap()
    with tile.TileContext(nc) as tc:
        build_body(tc, x_ap, w_ap, y_ap, s_sh)
    nc.compile()
    return nc


_NC_CACHE = {}


def _get_nc(s_sh):
    if s_sh not in _NC_CACHE:
        _NC_CACHE[s_sh] = build_nc(s_sh)
    return _NC_CACHE[s_sh]


def run(cached_states, W_u, trace=False, trace_cores=None):
    s_sh = S // N_CORES
    nc = _get_nc(s_sh)
    xs = np.asarray(cached_states, dtype=np.float32)
    ws = np.ascontiguousarray(np.asarray(W_u, dtype=np.float32))
    in_maps = []
    for i in range(N_CORES):
        sh = np.ascontiguousarray(xs[:, :, i * s_sh:(i + 1) * s_sh, :])
        in_maps.append({"x": sh, "w": ws})
    res = bass_utils.run_bass_kernel_spmd(
        nc, in_maps, core_ids=list(range(N_CORES)), trace=trace,
        trace_cores=trace_cores)
    out = np.empty((B, S, C), np.float32)
    for i in range(N_CORES):
        out[:, i * s_sh:(i + 1) * s_sh, :] = res.results[i]["y"]
    return out, res


def kernel(cached_states, W_u):
    out, _ = run(cached_states, W_u)
    return out
